# revision 1
# baseline (speedup 1.0000x reference)
"""Gaussian (norm) kernel matrix on 8 Trainium2 NeuronCores.

out[b, p] = exp(-sqrt(||x_b - proto_p||^2) / (2*sigma^2)), sigma = 1.

Sharding: x split along batch across 8 cores (1024 rows each); prototypes
replicated. Each core computes its [1024, 8192] slice.

Per-core math: d2 = x_sq + p_sq - 2*x.p is produced directly in PSUM by an
augmented matmul: K=256 cross term in bf16 (two K=128 accumulating matmuls
over (-2*p)^T) plus one K=4 bf16 matmul whose rows carry hi/lo bf16 splits
of p_sq and x_sq against ones (hi/lo restores ~fp32 precision of the norm
terms). ScalarE then does dist = sqrt(PSUM) and out = exp(-0.5*dist + 10*ln2)
written as fp16 (the 2^10 pre-scale keeps the tiny outputs in fp16 normal
range; the host divides it back out exactly). sqrt/exp live in different ACT
table sets (~2.7us per switch), so b-tiles run in two phases of 4 and the
ACT instruction order is pinned with scheduling-only deps — without the pin
the Tile scheduler interleaves sqrt/exp and quadruples the table loads.
"""

import numpy as np

import concourse.bacc as bacc
import concourse.mybir as mybir
import concourse.tile as tile
from concourse.tile import add_dep_helper
from concourse.bass_utils import run_bass_kernel_spmd

N_CORES = 8
B, P, F = 8192, 8192, 256
BS = B // N_CORES  # 1024 batch rows per core
BT = 128  # batch tile (partition dim)
NB = BS // BT  # 8 batch tiles per core
NCHUNK = 2048  # PSUM tile free size (4 banks; 2 bufs = all 8)
PHASE = 4  # b-tiles per activation-table phase
OUT_SCALE_LOG2 = 10  # exp pre-scale: out16 = 2^10 * exp(-dist/2)
F32 = mybir.dt.float32
F16 = mybir.dt.float16
BF16 = mybir.dt.bfloat16
AF = mybir.ActivationFunctionType


def build_nc(n_iters: int = 1):
    nc = bacc.Bacc("TRN2", target_bir_lowering=False, debug=False,
                   num_devices=N_CORES)
    xT_d = nc.dram_tensor("xT", [2, BT, BS], BF16, kind="ExternalInput")
    pT_d = nc.dram_tensor("pT", [2, BT, P], BF16, kind="ExternalInput")
    augL_d = nc.dram_tensor("augL", [4, BS], BF16, kind="ExternalInput")
    augR_d = nc.dram_tensor("augR", [4, P], BF16, kind="ExternalInput")
    out_d = nc.dram_tensor("out", [BS, P], F16, kind="ExternalOutput")

    with tile.TileContext(nc) as tc:
        with (
            tc.tile_pool(name="const", bufs=1) as cpool,
            tc.tile_pool(name="dist", bufs=PHASE) as dpool,
            tc.tile_pool(name="o16", bufs=2) as opool,
            tc.tile_pool(name="psum", bufs=2, space="PSUM") as ppool,
        ):
            xT = [cpool.tile([BT, BS], BF16, name=f"xT{k}", tag=f"xT{k}")
                  for k in range(2)]
            pT = [cpool.tile([BT, P], BF16, name=f"pT{k}", tag=f"pT{k}")
                  for k in range(2)]
            augL = cpool.tile([4, BS], BF16)
            augR = cpool.tile([4, P], BF16)
            ebias = cpool.tile([BT, 1], F32)
            nc.gpsimd.memset(ebias[:], float(OUT_SCALE_LOG2 * np.log(2.0)))
            for k in range(2):
                nc.sync.dma_start(xT[k][:], xT_d[k])
            nc.sync.dma_start(augL[:], augL_d[:])
            nc.sync.dma_start(augR[:], augR_d[:])
            for k in range(2):
                nc.sync.dma_start(pT[k][:], pT_d[k])

            acts = []  # pinned ACT execution order
            for _ in range(n_iters):
                for p0 in range(0, NB, PHASE):
                    bts = range(p0, min(p0 + PHASE, NB))
                    dists = {}
                    for bt in bts:
                        dist = dpool.tile([BT, P], F32)
                        dists[bt] = dist
                        bsl = slice(bt * BT, (bt + 1) * BT)
                        for c0 in range(0, P, NCHUNK):
                            ps = ppool.tile([BT, NCHUNK], F32)
                            for j in range(0, NCHUNK, 512):
                                n = slice(c0 + j, c0 + j + 512)
                                o = ps[:, j:j + 512]
                                nc.tensor.matmul(o, xT[0][:, bsl], pT[0][:, n],
                                                 start=True, stop=False)
                                nc.tensor.matmul(o, xT[1][:, bsl], pT[1][:, n],
                                                 start=False, stop=False)
                                nc.tensor.matmul(o, augL[:, bsl], augR[:, n],
                                                 start=False, stop=True)
                            acts.append(nc.scalar.activation(
                                dist[:, c0:c0 + NCHUNK], ps[:], AF.Sqrt))
                    for bt in bts:
                        for h0 in range(0, P, P // 2):
                            o16 = opool.tile([BT, P // 2], F16)
                            acts.append(nc.scalar.activation(
                                o16[:], dists[bt][:, h0:h0 + P // 2],
                                AF.Exp, scale=-0.5, bias=ebias[:]))
                            nc.sync.dma_start(
                                out_d[bt * BT:(bt + 1) * BT, h0:h0 + P // 2],
                                o16[:])
            for a, b in zip(acts, acts[1:]):
                add_dep_helper(b.ins, a.ins, sync=False,
                               reason="pin ACT order for table-set grouping")
    nc.compile()
    return nc


def _prep_inputs(x: np.ndarray, prototypes: np.ndarray):
    """Host-side shard + layout prep. Returns per-core in_maps."""
    bf16 = mybir.dt.np(BF16)
    x = np.ascontiguousarray(x, dtype=np.float32)
    p = np.ascontiguousarray(prototypes, dtype=np.float32)

    x_sq = np.sum(x * x, axis=-1)  # [B]
    p_sq = np.sum(p * p, axis=-1)  # [P]

    def hilo(v):
        hi = v.astype(bf16)
        lo = (v - hi.astype(np.float32)).astype(bf16)
        return hi, lo

    psq_hi, psq_lo = hilo(p_sq)
    ones_p = np.ones([P], dtype=bf16)
    augR = np.stack([psq_hi, psq_lo, ones_p, ones_p])  # [4, P]

    # [2, BT, P] with pT[k, r, n] = -2 * p[n, k*128 + r]
    pT = np.ascontiguousarray((-2.0 * p).T.reshape(2, BT, P)).astype(bf16)

    in_maps = []
    for c in range(N_CORES):
        xc = x[c * BS:(c + 1) * BS]  # [BS, F]
        xT = np.ascontiguousarray(xc.T.reshape(2, BT, BS)).astype(bf16)
        xsq_hi, xsq_lo = hilo(x_sq[c * BS:(c + 1) * BS])
        ones_b = np.ones([BS], dtype=bf16)
        augL = np.stack([ones_b, ones_b, xsq_hi, xsq_lo])  # [4, BS]
        in_maps.append({"xT": xT, "pT": pT, "augL": augL, "augR": augR})
    return in_maps


def _gather(per_core_outs):
    """fp16 shards -> fp32 full output, undoing the exact 2^10 pre-scale."""
    out = np.concatenate(per_core_outs, axis=0).astype(np.float32)
    out *= np.float32(2.0 ** -OUT_SCALE_LOG2)
    return out


def kernel(x: np.ndarray, prototypes: np.ndarray) -> np.ndarray:
    nc = build_nc()
    in_maps = _prep_inputs(x, prototypes)
    res = run_bass_kernel_spmd(nc, in_maps, list(range(N_CORES)))
    return _gather([res.results[c]["out"] for c in range(N_CORES)])



# revision 2
# speedup vs baseline: 21.3572x; 21.3572x over previous
"""Gaussian (norm) kernel matrix on 8 Trainium2 NeuronCores.

out[b, p] = exp(-sqrt(||x_b - proto_p||^2) / (2*sigma^2)), sigma = 1.

Sharding: x split along batch across 8 cores (1024 rows each); prototypes
replicated. Each core computes its [1024, 8192] slice.

Per-core math: d2 = x_sq + p_sq - 2*x.p is produced directly in PSUM by an
augmented matmul: K=256 cross term in bf16 (two K=128 accumulating matmuls
over (-2*p)^T) plus one K=4 bf16 matmul whose rows carry hi/lo bf16 splits
of p_sq and x_sq against ones (hi/lo restores ~fp32 precision of the norm
terms). ScalarE then does dist = sqrt(PSUM) and out = exp(-0.5*dist + 10*ln2)
written as fp16 (the 2^10 pre-scale keeps the tiny outputs in fp16 normal
range; the host divides it back out exactly). sqrt/exp live in different ACT
table sets (~2.7us per switch), so b-tiles run in two phases of 4 and the
ACT instruction order is pinned with scheduling-only deps — without the pin
the Tile scheduler interleaves sqrt/exp and quadruples the table loads.
"""

import numpy as np

import concourse.bacc as bacc
import concourse.mybir as mybir
import concourse.tile as tile
from concourse.tile import add_dep_helper
from concourse.bass_utils import run_bass_kernel_spmd

N_CORES = 8
B, P, F = 8192, 8192, 256
BS = B // N_CORES  # 1024 batch rows per core
BT = 128  # batch tile (partition dim)
NB = BS // BT  # 8 batch tiles per core
NCHUNK = 2048  # PSUM tile free size (4 banks; 2 bufs = all 8)
PHASE = 4  # b-tiles per activation-table phase
OUT_SCALE_LOG2 = 10  # exp pre-scale: out16 = 2^10 * exp(-dist/2)
F32 = mybir.dt.float32
F16 = mybir.dt.float16
BF16 = mybir.dt.bfloat16
AF = mybir.ActivationFunctionType


def build_nc(n_iters: int = 1):
    nc = bacc.Bacc("TRN2", target_bir_lowering=False, debug=False,
                   num_devices=N_CORES)
    xT_d = nc.dram_tensor("xT", [2, BT, BS], BF16, kind="ExternalInput")
    pT_d = nc.dram_tensor("pT", [2, BT, P], BF16, kind="ExternalInput")
    augL_d = nc.dram_tensor("augL", [4, BS], BF16, kind="ExternalInput")
    augR_d = nc.dram_tensor("augR", [4, P], BF16, kind="ExternalInput")
    out_d = nc.dram_tensor("out", [BS, P], F16, kind="ExternalOutput")

    from contextlib import ExitStack, nullcontext

    with tile.TileContext(nc) as tc:
        with (
            tc.tile_pool(name="const", bufs=1) as cpool,
            tc.tile_pool(name="dist", bufs=PHASE) as dpool,
            tc.tile_pool(name="o16", bufs=2) as opool,
            tc.tile_pool(name="psum", bufs=2, space="PSUM") as ppool,
        ):
            xT = [cpool.tile([BT, BS], BF16, name=f"xT{k}", tag=f"xT{k}")
                  for k in range(2)]
            pT = [cpool.tile([BT, P], BF16, name=f"pT{k}", tag=f"pT{k}")
                  for k in range(2)]
            augL = cpool.tile([4, BS], BF16)
            augR = cpool.tile([4, P], BF16)
            ebias = cpool.tile([BT, 1], F32)
            nc.gpsimd.memset(ebias[:], float(OUT_SCALE_LOG2 * np.log(2.0)))
            for k in range(2):
                nc.sync.dma_start(xT[k][:], xT_d[k])
            nc.sync.dma_start(augL[:], augL_d[:])
            nc.sync.dma_start(augR[:], augR_d[:])
            for k in range(2):
                nc.sync.dma_start(pT[k][:], pT_d[k])

            # n_iters>1 (timing builds): HW loop repeats the identical body
            loop = tc.For_i(0, n_iters) if n_iters > 1 else nullcontext()
            acts = []  # pinned ACT execution order
            with loop:
                for p0 in range(0, NB, PHASE):
                    bts = range(p0, min(p0 + PHASE, NB))
                    dists = {}
                    for bt in bts:
                        dist = dpool.tile([BT, P], F32)
                        dists[bt] = dist
                        bsl = slice(bt * BT, (bt + 1) * BT)
                        for c0 in range(0, P, NCHUNK):
                            ps = ppool.tile([BT, NCHUNK], F32)
                            for j in range(0, NCHUNK, 512):
                                n = slice(c0 + j, c0 + j + 512)
                                o = ps[:, j:j + 512]
                                nc.tensor.matmul(o, xT[0][:, bsl], pT[0][:, n],
                                                 start=True, stop=False)
                                nc.tensor.matmul(o, xT[1][:, bsl], pT[1][:, n],
                                                 start=False, stop=False)
                                nc.tensor.matmul(o, augL[:, bsl], augR[:, n],
                                                 start=False, stop=True)
                            acts.append(nc.scalar.activation(
                                dist[:, c0:c0 + NCHUNK], ps[:], AF.Sqrt))
                    for bt in bts:
                        for h0 in range(0, P, P // 2):
                            o16 = opool.tile([BT, P // 2], F16)
                            acts.append(nc.scalar.activation(
                                o16[:], dists[bt][:, h0:h0 + P // 2],
                                AF.Exp, scale=-0.5, bias=ebias[:]))
                            nc.sync.dma_start(
                                out_d[bt * BT:(bt + 1) * BT, h0:h0 + P // 2],
                                o16[:])
            for a, b in zip(acts, acts[1:]):
                add_dep_helper(b.ins, a.ins, sync=False,
                               reason="pin ACT order for table-set grouping")
    nc.compile()
    return nc


def _prep_inputs(x: np.ndarray, prototypes: np.ndarray):
    """Host-side shard + layout prep. Returns per-core in_maps."""
    bf16 = mybir.dt.np(BF16)
    x = np.ascontiguousarray(x, dtype=np.float32)
    p = np.ascontiguousarray(prototypes, dtype=np.float32)

    x_sq = np.sum(x * x, axis=-1)  # [B]
    p_sq = np.sum(p * p, axis=-1)  # [P]

    def hilo(v):
        hi = v.astype(bf16)
        lo = (v - hi.astype(np.float32)).astype(bf16)
        return hi, lo

    psq_hi, psq_lo = hilo(p_sq)
    ones_p = np.ones([P], dtype=bf16)
    augR = np.stack([psq_hi, psq_lo, ones_p, ones_p])  # [4, P]

    # [2, BT, P] with pT[k, r, n] = -2 * p[n, k*128 + r]
    pT = np.ascontiguousarray((-2.0 * p).T.reshape(2, BT, P)).astype(bf16)

    in_maps = []
    for c in range(N_CORES):
        xc = x[c * BS:(c + 1) * BS]  # [BS, F]
        xT = np.ascontiguousarray(xc.T.reshape(2, BT, BS)).astype(bf16)
        xsq_hi, xsq_lo = hilo(x_sq[c * BS:(c + 1) * BS])
        ones_b = np.ones([BS], dtype=bf16)
        augL = np.stack([ones_b, ones_b, xsq_hi, xsq_lo])  # [4, BS]
        in_maps.append({"xT": xT, "pT": pT, "augL": augL, "augR": augR})
    return in_maps


def _gather(per_core_outs):
    """fp16 shards -> fp32 full output, undoing the exact 2^10 pre-scale."""
    out = np.concatenate(per_core_outs, axis=0).astype(np.float32)
    out *= np.float32(2.0 ** -OUT_SCALE_LOG2)
    return out


def kernel(x: np.ndarray, prototypes: np.ndarray) -> np.ndarray:
    nc = build_nc()
    in_maps = _prep_inputs(x, prototypes)
    res = run_bass_kernel_spmd(nc, in_maps, list(range(N_CORES)))
    return _gather([res.results[c]["out"] for c in range(N_CORES)])

